# revision 1
# baseline (speedup 1.0000x reference)
"""Trainium2 Bass kernel for nn_DescriptionAware (dense_mlp).

Self-contained: takes FULL inputs (as in reference.setup_inputs()), shards
across 8 NeuronCores (batch x class-half), runs one SPMD Bass/Tile program,
reassembles the full [B,S,C] f32 logits on host.

Sharding: core k handles batch b=k//2 and classes [32*(k%2), 32*(k%2)+32).
Matmul operands use float32r (tf32-like, 1 cyc/row at N>=256, ~1e-4 rel err).
"""

import os
import numpy as np

import concourse.bass as bass
import concourse.mybir as mybir
import concourse.tile as tile
from concourse import bacc
from concourse.bass import IndirectOffsetOnAxis
from concourse.bass_utils import run_bass_kernel_spmd
from concourse.tile_rust import add_dep_helper

# problem dims (hardcoded per contract)
B, S, H = 4, 256, 768
C = 64
LD = 128
E = 300
NS = 8
LP = 32
LA = 16
V = 50000
DH = 300

NCORES = 8
CH = 32                      # classes per core
DCH = [(0, 128), (128, 256), (256, 300)]   # d-chunks of DH=300
HCH = 6                      # 768 = 6*128
KL = [(768, 896), (896, 1024), (1024, 1152), (1152, 1196)]  # W1l row chunks (abs rows in W1)
KP = [(1196 + 128 * i, 1196 + 128 * (i + 1)) for i in range(6)]  # W1p rows
KA = [(128 * i, min(128 * (i + 1), 1069)) for i in range(9)]     # Wa1_aug row chunks

# chunk order: gather-DMA k covers class-block k (all 8 senses)
GORDER = [g for cb in range(4) for g in range(cb, 32, 4)]

F32 = mybir.dt.float32
F32R = mybir.dt.float32r
BF16 = mybir.dt.bfloat16
I32 = mybir.dt.int32
AL = mybir.AluOpType
AF = mybir.ActivationFunctionType

# C128 const column layout
PIOTA0, PIOTA1 = 0, 1
P16_0 = 2          # 8 cols
P32A_0 = 10        # 8 cols: delta(p//32 == j) for j<4 else 0
P32B_0 = 18        # 8 cols: delta(p//32 == j-4) for j>=4 else 0
ID_0 = 26          # 128 cols
L8_0 = 154         # 128 cols
BLK_0 = 282        # 16 cols
ONESB_0 = 298      # 128 cols, all ones
NC128 = 426
# C8 const column layout
EXPW0_0 = 0        # 128
EXPW1_0 = 128      # 128
L4_0 = 256         # 128
BLK4_0 = 384       # 2
ONES_0 = 386       # 1
NC8 = 387


def _host_consts():
    c128 = np.zeros((128, NC128), np.float32)
    p = np.arange(128)
    c128[:, PIOTA0] = p
    c128[:, PIOTA1] = p + 128
    for j in range(8):
        c128[p // 16 == j, P16_0 + j] = 1.0
    for j in range(4):
        c128[p // 32 == j, P32A_0 + j] = 1.0
        c128[p // 32 == j, P32B_0 + 4 + j] = 1.0
    c128[:, ID_0:ID_0 + 128] = np.eye(128, dtype=np.float32)
    q = np.arange(128)
    c128[:, L8_0:L8_0 + 128] = (q[:, None] % 8 == (np.arange(128)[None, :] // 16)).astype(np.float32)
    gc = np.arange(16)
    fg = 4 * (gc // 4) + (gc % 4)
    c128[:, BLK_0:BLK_0 + 16] = (q[:, None] // 8 == fg[None, :]).astype(np.float32)
    c128[:, ONESB_0:ONESB_0 + 128] = 1.0

    c8 = np.zeros((8, NC8), np.float32)
    k = np.arange(8)
    c8[:, EXPW0_0:EXPW0_0 + 128] = (k[:, None] == (np.arange(128)[None, :] // 32)).astype(np.float32)
    c8[:, EXPW1_0:EXPW1_0 + 128] = (k[:, None] == 4 + (np.arange(128)[None, :] // 32)).astype(np.float32)
    c8[:, L4_0:L4_0 + 128] = (k[:, None] % 4 == (np.arange(128)[None, :] // 32)).astype(np.float32)
    c8[:, BLK4_0:BLK4_0 + 2] = (k[:, None] // 4 == np.arange(2)[None, :]).astype(np.float32)
    c8[:, ONES_0] = 1.0
    return c128, c8


def build_program():
    nc = bacc.Bacc("TRN2", target_bir_lowering=False, debug=False, num_devices=NCORES,
                   dynamic_dma_scratch_size=65536)

    dt = nc.dram_tensor
    t_x = dt("x", [128, 2 * H], F32R, kind="ExternalInput")
    t_wemb = dt("wemb", [V, E], F32R, kind="ExternalInput")
    t_aidxn = dt("aidxn", [128, 2 * LA], I32, kind="ExternalInput")
    t_aidxg = dt("aidxg", [128, 32], I32, kind="ExternalInput")
    t_pidxn = dt("pidxn", [NS, LP], I32, kind="ExternalInput")
    t_pidxg = dt("pidxg", [128, 2], I32, kind="ExternalInput")
    t_pse = dt("pse", [1, 2], I32, kind="ExternalInput")
    t_lembT = dt("lembT", [LD, CH], F32R, kind="ExternalInput")
    t_wa1 = dt("wa1", [128, 9 * H], F32R, kind="ExternalInput")
    t_wa2r = dt("wa2r", [1, H], F32, kind="ExternalInput")
    t_w1x = dt("w1x", [128, 6 * DH], F32R, kind="ExternalInput")
    t_w1l = dt("w1l", [128, 4 * DH], F32R, kind="ExternalInput")
    t_w1p = dt("w1p", [128, 6 * DH], F32R, kind="ExternalInput")
    t_w2 = dt("w2", [128, 3 * 32], BF16, kind="ExternalInput")
    t_b1r = dt("b1r", [1, DH], F32, kind="ExternalInput")
    t_b2 = dt("b2", [1, 1], F32, kind="ExternalInput")
    t_ba2 = dt("ba2", [1, 1], F32, kind="ExternalInput")
    t_c128 = dt("c128", [128, NC128], F32R, kind="ExternalInput")
    t_c8 = dt("c8", [8, NC8], F32R, kind="ExternalInput")
    t_out = dt("out", [16, 512], F32, kind="ExternalOutput")

    with tile.TileContext(nc) as tc:
        with tc.tile_pool(name="sb", bufs=1) as sb, \
             tc.tile_pool(name="sbt", bufs=6) as sbt, \
             tc.tile_pool(name="ppw", bufs=3, space="PSUM") as ppw, \
             tc.tile_pool(name="ppa", bufs=2, space="PSUM") as ppa, \
             tc.tile_pool(name="ppo", bufs=1, space="PSUM") as ppo:

            # ---------------- DMAs (order = per-engine issue order) ----------------
            # sync (HWDGE) queue: gather indices first, then big weights
            aidxg = sb.tile([128, 32], I32, tag="aidxg")
            nc.sync.dma_start(out=aidxg[:], in_=t_aidxg[:])
            pidxg = sb.tile([128, 2], I32, tag="pidxg")
            nc.sync.dma_start(out=pidxg[:], in_=t_pidxg[:])
            c8 = sb.tile([8, NC8], F32R, tag="c8")
            nc.sync.dma_start(out=c8[:], in_=t_c8[:])
            c128 = sb.tile([128, NC128], F32R, tag="c128")
            nc.sync.dma_start(out=c128[:], in_=t_c128[:])
            wa1_all = sb.tile([128, 9 * H], F32R, tag="wa1_all")
            nc.sync.dma_start(out=wa1_all[:], in_=t_wa1[:])
            wa1 = [wa1_all[0:(r1 - r0), H * i:H * (i + 1)] for i, (r0, r1) in enumerate(KA)]
            x_all = sb.tile([128, 2 * H], F32R, tag="x_all")
            nc.sync.dma_start(out=x_all[:], in_=t_x[:])
            xt = [x_all[:, H * st:H * (st + 1)] for st in range(2)]
            w1x_all = sb.tile([128, 6 * DH], F32R, tag="w1x_all")
            nc.sync.dma_start(out=w1x_all[:], in_=t_w1x[:])
            w1x = [w1x_all[:, DH * hc:DH * (hc + 1)] for hc in range(HCH)]

            # gpsimd queue: gathers first, tiny cast DMA after
            # gathers (HW indirect DMA only honors [128,1] offset tables)
            pdG = sb.tile([128, 2 * E], F32R, tag="pdG")
            for h in range(2):
                nc.gpsimd.indirect_dma_start(
                    out=pdG[:, E * h:E * (h + 1)], out_offset=None, in_=t_wemb[:],
                    in_offset=IndirectOffsetOnAxis(ap=pidxg[:, h:h + 1], axis=0))
            argG = []
            gather_insts = []
            for k in range(4):
                g_ = sb.tile([128, 8 * E], F32R, tag=f"argG{k}")
                for j in range(8):
                    gi = nc.gpsimd.indirect_dma_start(
                        out=g_[:, E * j:E * (j + 1)], out_offset=None, in_=t_wemb[:],
                        in_offset=IndirectOffsetOnAxis(ap=aidxg[:, 8 * k + j:8 * k + j + 1],
                                                       axis=0))
                    gather_insts.append(gi)
                argG.append(g_)

            sef = sb.tile([1, 2], F32, tag="sef")
            nc.gpsimd.dma_start(out=sef[:], in_=t_pse[:])  # int32 -> f32 cast

            # scalar (HWDGE) queue: the rest
            wa2r = sb.tile([1, H], F32, tag="wa2r")
            nc.scalar.dma_start(out=wa2r[:], in_=t_wa2r[:])
            b2t = sb.tile([1, 1], F32, tag="b2t")
            nc.scalar.dma_start(out=b2t[:], in_=t_b2[:])
            ba2t = sb.tile([1, 1], F32, tag="ba2t")
            nc.scalar.dma_start(out=ba2t[:], in_=t_ba2[:])
            pidxn = sb.tile([8, LP], I32, tag="pidxn")
            nc.scalar.dma_start(out=pidxn[:], in_=t_pidxn[:])
            aidxn_all = sb.tile([128, 2 * LA], I32, tag="aidxn_all")
            nc.scalar.dma_start(out=aidxn_all[:], in_=t_aidxn[:])
            aidxn = [aidxn_all[:, LA * tg:LA * (tg + 1)] for tg in range(2)]
            lembT = sb.tile([LD, CH], F32R, tag="lembT")
            nc.scalar.dma_start(out=lembT[:], in_=t_lembT[:])
            w2_all = sb.tile([128, 3 * 32], BF16, tag="w2_all")
            nc.scalar.dma_start(out=w2_all[:], in_=t_w2[:])
            w2c = [w2_all[0:(d1 - d0), 32 * i:32 * (i + 1)] for i, (d0, d1) in enumerate(DCH)]
            b1r = sb.tile([1, DH], F32, tag="b1r")
            nc.scalar.dma_start(out=b1r[:], in_=t_b1r[:])
            w1l_all = sb.tile([128, 4 * DH], F32R, tag="w1l_all")
            nc.scalar.dma_start(out=w1l_all[:], in_=t_w1l[:])
            w1l = [w1l_all[0:(r1 - r0), DH * i:DH * (i + 1)] for i, (r0, r1) in enumerate(KL)]
            w1p_all = sb.tile([128, 6 * DH], F32R, tag="w1p_all")
            nc.scalar.dma_start(out=w1p_all[:], in_=t_w1p[:])
            w1p = [w1p_all[:, DH * i:DH * (i + 1)] for i in range(6)]


            # ---------------- small prep ----------------
            # broadcasts via tiny PE matmuls (keep gpsimd free for gather emission)
            ones128r = c128[0:1, ONESB_0:ONESB_0 + 128].bitcast(F32)
            ones8r = c8[0:1, EXPW0_0:EXPW0_0 + 8].bitcast(F32)
            sebps = ppw.tile([128, 2], F32, tag="w", name="sebps")
            nc.tensor.matmul(out=sebps[:], lhsT=ones128r, rhs=sef[:], start=True, stop=True)
            seb = sb.tile([128, 2], F32, tag="seb")
            nc.vector.tensor_copy(out=seb[:], in_=sebps[:])
            wa2b = sb.tile([8, H], F32, tag="wa2b")
            for nb in range(2):
                wp_ = ppw.tile([8, 384], F32, tag="w", name=f"wa2ps{nb}")
                nc.tensor.matmul(out=wp_[:], lhsT=ones8r,
                                 rhs=wa2r[0:1, 384 * nb:384 * (nb + 1)], start=True, stop=True)
                nc.vector.tensor_copy(out=wa2b[:, 384 * nb:384 * (nb + 1)], in_=wp_[:])
            ba2ps = ppw.tile([8, 1], F32, tag="w", name="ba2ps")
            nc.tensor.matmul(out=ba2ps[:], lhsT=ones8r, rhs=ba2t[:], start=True, stop=True)
            ba2b = sb.tile([8, 1], F32, tag="ba2b")
            nc.vector.tensor_copy(out=ba2b[:], in_=ba2ps[:])
            b2ps = ppw.tile([128, 1], F32, tag="w", name="b2ps")
            nc.tensor.matmul(out=b2ps[:], lhsT=ones128r, rhs=b2t[:], start=True, stop=True)
            b2b = sb.tile([128, 1], F32, tag="b2b")
            nc.vector.tensor_copy(out=b2b[:], in_=b2ps[:])

            # span mask
            m1 = sb.tile([128, 2], F32, tag="m1")
            nc.vector.tensor_scalar(out=m1[:], in0=c128[:, PIOTA0:PIOTA0 + 2],
                                    scalar1=seb[:, 0:1], scalar2=None, op0=AL.is_ge)
            m2 = sb.tile([128, 2], F32, tag="m2")
            nc.vector.tensor_scalar(out=m2[:], in0=c128[:, PIOTA0:PIOTA0 + 2],
                                    scalar1=seb[:, 1:2], scalar2=None, op0=AL.is_lt)
            smask = sb.tile([128, 2], F32R, tag="smask")
            nc.vector.tensor_tensor(out=smask[:], in0=m1[:], in1=m2[:], op=AL.mult)
            dlen = sb.tile([1, 1], F32, tag="dlen")
            nc.vector.tensor_tensor(out=dlen[:], in0=sef[:, 1:2], in1=sef[:, 0:1], op=AL.subtract)
            dlm = sb.tile([1, 1], F32, tag="dlm")
            nc.vector.tensor_scalar(out=dlm[:], in0=dlen[:], scalar1=1.0, scalar2=None, op0=AL.max)
            rspl = sb.tile([1, 1], F32, tag="rspl")
            nc.vector.reciprocal(out=rspl[:], in_=dlm[:])

            # arg lens / recip (per (n,c) row, 2 tiles)
            rlen = []
            for tgt in range(2):
                fI = sbt.tile([128, LA], F32, tag="fI")
                nc.vector.tensor_copy(out=fI[:], in_=aidxn[tgt][:])
                mk = sbt.tile([128, LA], F32, tag="mk")
                nc.vector.tensor_scalar(out=mk[:], in0=fI[:], scalar1=0.5, scalar2=None, op0=AL.is_ge)
                ln = sbt.tile([128, 1], F32, tag="ln")
                nc.vector.tensor_reduce(out=ln[:], in_=mk[:], axis=mybir.AxisListType.X, op=AL.add)
                lnm = sbt.tile([128, 1], F32, tag="lnm")
                nc.vector.tensor_scalar(out=lnm[:], in0=ln[:], scalar1=1.0, scalar2=None, op0=AL.max)
                r_ = sb.tile([128, 1], F32, tag=f"rlen{tgt}")
                nc.vector.reciprocal(out=r_[:], in_=lnm[:])
                rlen.append(r_)

            # pd lens
            pf = sb.tile([8, LP], F32, tag="pf")
            nc.vector.tensor_copy(out=pf[:], in_=pidxn[:])
            pmk = sb.tile([8, LP], F32, tag="pmk")
            nc.vector.tensor_scalar(out=pmk[:], in0=pf[:], scalar1=0.5, scalar2=None, op0=AL.is_ge)
            plen = sb.tile([8, 1], F32, tag="plen")
            nc.vector.tensor_reduce(out=plen[:], in_=pmk[:], axis=mybir.AxisListType.X, op=AL.add)
            scol = sb.tile([8, 1], F32, tag="scol")
            nc.vector.tensor_scalar(out=scol[:], in0=plen[:], scalar1=0.5, scalar2=-100000.0,
                                    op0=AL.is_lt, op1=AL.mult)
            plm = sb.tile([8, 1], F32, tag="plm")
            nc.vector.tensor_scalar(out=plm[:], in0=plen[:], scalar1=1.0, scalar2=None, op0=AL.max)
            rp8 = sb.tile([8, 1], F32, tag="rp8")
            nc.vector.reciprocal(out=rp8[:], in_=plm[:])

            # masks in gather layout
            agf = sb.tile([128, 32], F32, tag="agf")
            nc.vector.tensor_copy(out=agf[:], in_=aidxg[:])
            maG = sb.tile([128, 32], F32, tag="maG")
            nc.vector.tensor_scalar(out=maG[:], in0=agf[:], scalar1=0.5, scalar2=None, op0=AL.is_ge)
            pgf = sb.tile([128, 2], F32, tag="pgf")
            nc.vector.tensor_copy(out=pgf[:], in_=pidxg[:])
            pmG = sb.tile([128, 2], F32, tag="pmG")
            nc.vector.tensor_scalar(out=pmG[:], in0=pgf[:], scalar1=0.5, scalar2=None, op0=AL.is_ge)

            # rpx[p,h] = rp8[4h + p//32]
            rhs4 = sb.tile([8, 2], F32R, tag="rhs4")
            nc.vector.tensor_scalar(out=rhs4[:], in0=c8[:, BLK4_0:BLK4_0 + 2],
                                    scalar1=rp8[:], scalar2=None, op0=AL.mult)
            rpxp = ppw.tile([128, 2], F32, tag="w")
            nc.tensor.matmul(out=rpxp[:], lhsT=c8[:, L4_0:L4_0 + 128], rhs=rhs4[:],
                             start=True, stop=True)
            rpx = sb.tile([128, 2], F32, tag="rpx")
            nc.vector.tensor_copy(out=rpx[:], in_=rpxp[:])

            # ---------------- xT + hxT ----------------
            ident = c128[:, ID_0:ID_0 + 128]
            xT = []
            for hc in range(HCH):
                xTh = sb.tile([128, S], F32R, tag=f"xT{hc}")
                xT.append(xTh)
            for st in range(2):
                for hc in range(HCH):
                    tp = ppw.tile([128, 128], F32R, tag="w", name="tp")
                    nc.tensor.transpose(out=tp[:], in_=xt[st][:, 128 * hc:128 * (hc + 1)],
                                        identity=ident[:, :])
                    nc.vector.tensor_copy(out=xT[hc][:, 128 * st:128 * (st + 1)], in_=tp[:])

            hxT = []
            for dc, (d0, d1) in enumerate(DCH):
                ds_ = d1 - d0
                hp_ = ppw.tile([ds_, S], F32, tag="w", name="hp_")
                for hc in range(HCH):
                    nc.tensor.matmul(out=hp_[:], lhsT=w1x[hc][:, d0:d1], rhs=xT[hc][:],
                                     start=(hc == 0), stop=(hc == HCH - 1))
                hs = sb.tile([ds_, S], F32, tag=f"hxT{dc}")
                nc.scalar.copy(out=hs[:], in_=hp_[:])
                hxT.append(hs)

            # ---------------- pred_agg row + predT ----------------
            prow_h = [ppw.tile([1, 384], F32, tag="w", name=f"prow{nb}") for nb in range(2)]
            for nb in range(2):
                for st in range(2):
                    nc.tensor.matmul(out=prow_h[nb][:],
                                     lhsT=smask[:, st:st + 1],
                                     rhs=xt[st][:, 384 * nb:384 * (nb + 1)],
                                     start=(st == 0), stop=(st == 1), tile_position=(0, 0))
            prow_s = sb.tile([1, H], F32R, tag="prow_s")
            for nb in range(2):
                nc.scalar.activation(out=prow_s[0:1, 384 * nb:384 * (nb + 1)],
                                     in_=prow_h[nb][:], func=AF.Copy, scale=rspl[:, :])
            predT = []
            for hc in range(HCH):
                tp = ppw.tile([128, 1], F32R, tag="w", name="tpp")
                nc.tensor.transpose(out=tp[:].bitcast(F32),
                                    in_=prow_s[0:1, 128 * hc:128 * (hc + 1)].bitcast(F32),
                                    identity=ident[0:1, 0:1].bitcast(F32))
                pt = sb.tile([128, 1], F32R, tag=f"predT{hc}")
                nc.vector.tensor_copy(out=pt[:], in_=tp[:])
                predT.append(pt)

            # ---------------- pd_agg + pdT + att chunks ----------------
            pd_agg = sb.tile([8, E], F32R, tag="pd_agg")
            pdps = ppw.tile([8, E], F32, tag="w", name="pdps")
            for h in range(2):
                p32_0 = P32A_0 if h == 0 else P32B_0
                psel = sbt.tile([128, 8], F32R, tag="psel")
                nc.vector.tensor_scalar(out=psel[:], in0=c128[:, p32_0:p32_0 + 8],
                                        scalar1=pmG[:, h:h + 1], scalar2=rpx[:, h:h + 1],
                                        op0=AL.mult, op1=AL.mult)
                nc.tensor.matmul(out=pdps[:], lhsT=psel[:], rhs=pdG[:, E * h:E * (h + 1)],
                                 start=(h == 0), stop=(h == 1))
            nc.vector.tensor_copy(out=pd_agg[:], in_=pdps[:])

            attk = []
            for k in range(6):
                a_ = sb.tile([128, 8], F32R, tag=f"attk{k}", name=f"attk{k}")
                nc.vector.tensor_copy(out=a_[:], in_=predT[k][:, 0:1].to_broadcast([128, 8]))
                attk.append(a_)
            for e in range(2):
                tp = ppw.tile([128, 8], F32R, tag="w", name="tpa")
                nc.tensor.transpose(out=tp[:], in_=pd_agg[:, 128 * e:128 * (e + 1)],
                                    identity=ident[0:8, 0:8])
                a_ = sb.tile([128, 8], F32R, tag=f"attk{6 + e}", name=f"attk{6+e}")
                nc.vector.tensor_copy(out=a_[:], in_=tp[:])
                attk.append(a_)
            tp = ppw.tile([44, 8], F32R, tag="w", name="tpb")
            nc.tensor.transpose(out=tp[:], in_=pd_agg[:, 256:300], identity=ident[0:8, 0:8])
            a_ = sb.tile([45, 8], F32R, tag="attk8")
            nc.vector.memset(a_[:, :].bitcast(F32), 1.0)
            nc.vector.tensor_copy(out=a_[0:44, :], in_=tp[:])
            attk.append(a_)

            # ---------------- attention MLP -> weights col ----------------
            hidp = []
            for nb in range(2):
                hp2 = ppw.tile([8, 384], F32, tag="w", name=f"hid{nb}")
                for k in range(9):
                    nc.tensor.matmul(out=hp2[:], lhsT=attk[k][:],
                                     rhs=wa1[k][:, 384 * nb:384 * (nb + 1)],
                                     start=(k == 0), stop=(k == 8))
                hidp.append(hp2)
            hid = sb.tile([8, H], F32, tag="hid")
            for nb in range(2):
                nc.scalar.activation(out=hid[:, 384 * nb:384 * (nb + 1)], in_=hidp[nb][:],
                                     func=AF.Relu)
            scr = sb.tile([8, H], F32, tag="scr")
            nc.vector.tensor_tensor(out=scr[:], in0=hid[:], in1=wa2b[:], op=AL.mult)
            wraw = sb.tile([8, 1], F32, tag="wraw")
            nc.vector.tensor_reduce(out=wraw[:], in_=scr[:], axis=mybir.AxisListType.X,
                                    op=AL.add)
            wsb = sb.tile([8, 1], F32, tag="wsb")
            nc.vector.tensor_scalar(out=wsb[:], in0=wraw[:], scalar1=scol[:],
                                    scalar2=ba2b[:, :], op0=AL.add, op1=AL.add)
            expc = sb.tile([8, 1], F32R, tag="expc")
            nc.scalar.activation(out=expc[:], in_=wsb[:], func=AF.Exp)
            sps = ppw.tile([1, 1], F32, tag="w", name="sps")
            nc.tensor.matmul(out=sps[:], lhsT=expc[:].bitcast(F32),
                             rhs=c8[:, ONES_0:ONES_0 + 1].bitcast(F32),
                             start=True, stop=True)
            rs = sb.tile([1, 1], F32, tag="rs")
            nc.vector.reciprocal(out=rs[:], in_=sps[:])
            rbps = ppw.tile([8, 1], F32, tag="w", name="rbps")
            nc.tensor.matmul(out=rbps[:], lhsT=ones8r, rhs=rs[:], start=True, stop=True)
            wcol = sb.tile([8, 1], F32R, tag="wcol")
            nc.vector.tensor_tensor(out=wcol[:], in0=expc[:], in1=rbps[:], op=AL.mult)

            # ---------------- wr / wrx ----------------
            wrx = sb.tile([128, 32], F32, tag="wrx")
            for tgt in range(2):
                ew0 = EXPW0_0 if tgt == 0 else EXPW1_0
                wps = ppw.tile([128, 1], F32, tag="w", name=f"wps{tgt}")
                nc.tensor.matmul(out=wps[:], lhsT=c8[:, ew0:ew0 + 128].bitcast(F32),
                                 rhs=wcol[:].bitcast(F32), start=True, stop=True)
                wr_ = sbt.tile([128, 1], F32, tag="wr")
                nc.vector.tensor_tensor(out=wr_[:], in0=wps[:], in1=rlen[tgt][:], op=AL.mult)
                rhsW = sbt.tile([128, 16], F32R, tag="rhsW")
                nc.vector.tensor_scalar(out=rhsW[:], in0=c128[:, BLK_0:BLK_0 + 16],
                                        scalar1=wr_[:], scalar2=None, op0=AL.mult)
                wxp = ppw.tile([128, 16], F32, tag="w", name=f"wxp{tgt}")
                nc.tensor.matmul(out=wxp[:], lhsT=c128[:, L8_0:L8_0 + 128], rhs=rhsW[:],
                                 start=True, stop=True)
                nc.vector.tensor_copy(out=wrx[:, 16 * tgt:16 * (tgt + 1)], in_=wxp[:])

            # ---------------- hp row -> hpbT ----------------
            hprow = ppw.tile([1, DH], F32, tag="w", name="hprow")
            for i in range(6):
                nc.tensor.matmul(out=hprow[:], lhsT=predT[i][:], rhs=w1p[i][:],
                                 start=(i == 0), stop=(i == 5), tile_position=(0, 0))
            hpb = sb.tile([1, DH], F32R, tag="hpb")
            nc.vector.tensor_tensor(out=hpb[:], in0=hprow[:], in1=b1r[:], op=AL.add)
            hpbT = []
            for dc, (d0, d1) in enumerate(DCH):
                tp2 = ppw.tile([d1 - d0, 1], F32R, tag="w", name="tp2")
                nc.tensor.transpose(out=tp2[:].bitcast(F32),
                                    in_=hpb[0:1, d0:d1].bitcast(F32),
                                    identity=ident[0:1, 0:1].bitcast(F32))
                hb = sb.tile([d1 - d0, 1], F32, tag=f"hpbT{dc}")
                nc.vector.tensor_copy(out=hb[:], in_=tp2[:])
                hpbT.append(hb)

            # ---------------- per class-block: args -> hl -> bias -> final ----------------
            p16 = c128[:, P16_0:P16_0 + 8]
            prev_group_end = [None, None]
            outp2 = None
            for cb in range(4):
                cyc = cb // 2
                if cb % 2 == 0:
                    outp2 = [ppo.tile([128, 512], F32, tag=f"out{h}", name=f"outp{h}_{cyc}")
                             for h in range(2)]
                    prev_group_end = [None, None]
                aw = ppa.tile([8, E], F32, tag="acc", name="aw")
                for j in range(8):
                    i = 8 * cb + j
                    g = GORDER[i]
                    wsel = sbt.tile([128, 8], F32R, tag="wsel")
                    nc.vector.tensor_scalar(out=wsel[:], in0=p16,
                                            scalar1=maG[:, i:i + 1], scalar2=wrx[:, g:g + 1],
                                            op0=AL.mult, op1=AL.mult)
                    nc.tensor.matmul(out=aw[:], lhsT=wsel[:],
                                     rhs=argG[cb][:, E * j:E * (j + 1)],
                                     start=(j == 0), stop=(j == 7))
                aws = sbt.tile([8, E], F32R, tag="aws")
                nc.vector.tensor_copy(out=aws[:], in_=aw[:])

                liTa = []
                for e, (e0, e1) in enumerate(DCH):
                    tp3 = ppw.tile([e1 - e0, 8], F32R, tag="w", name="tp3")
                    nc.tensor.transpose(out=tp3[:], in_=aws[:, e0:e1], identity=ident[0:8, 0:8])
                    li = sbt.tile([e1 - e0, 8], F32R, tag=f"liTa{e}", name=f"li{e}")
                    nc.vector.tensor_copy(out=li[:], in_=tp3[:])
                    liTa.append(li)

                hl = ppw.tile([8, DH], F32, tag="w", name="hl")
                for kc in range(4):
                    lh = lembT[:, 8 * cb:8 * (cb + 1)] if kc == 0 else liTa[kc - 1][:]
                    nc.tensor.matmul(out=hl[:], lhsT=lh, rhs=w1l[kc][:],
                                     start=(kc == 0), stop=(kc == 3))
                hls = sbt.tile([8, DH], F32R, tag="hls")
                nc.vector.tensor_copy(out=hls[:], in_=hl[:])

                biasT = []
                for dc, (d0, d1) in enumerate(DCH):
                    tp4 = ppw.tile([d1 - d0, 8], F32R, tag="w", name="tp4")
                    nc.tensor.transpose(out=tp4[:], in_=hls[:, d0:d1], identity=ident[0:8, 0:8])
                    bt = sbt.tile([d1 - d0, 8], F32, tag=f"biasT{dc}", name=f"bt{dc}")
                    nc.vector.tensor_scalar(out=bt[:], in0=tp4[:], scalar1=hpbT[dc][:],
                                            scalar2=None, op0=AL.add)
                    biasT.append(bt)

                for cl in range(4):
                    cp = 4 * cb + cl
                    h = cp % 2
                    row = 32 * ((cp % 8) // 2)
                    for dc, (d0, d1) in enumerate(DCH):
                        ds_ = d1 - d0
                        tt = sbt.tile([ds_, 512], BF16, tag="t", name="tt")
                        nc.scalar.activation(out=tt[:, 0:256], in_=hxT[dc][:], func=AF.Relu,
                                             bias=biasT[dc][:, 2 * cl:2 * cl + 1])
                        nc.vector.tensor_scalar(out=tt[:, 256:512], in0=hxT[dc][:],
                                                scalar1=biasT[dc][:, 2 * cl + 1:2 * cl + 2],
                                                scalar2=0.0, op0=AL.add, op1=AL.max)
                        mm = nc.tensor.matmul(out=outp2[h][row:row + 32, :], lhsT=w2c[dc][:],
                                              rhs=tt[:], start=(dc == 0), stop=(dc == 2),
                                              tile_position=(0, row), skip_group_check=True)
                        if dc == 0 and prev_group_end[h] is not None:
                            add_dep_helper(mm.ins, prev_group_end[h], sync=False,
                                           reason="serialize psum accumulation groups per bank")
                        if dc == 2:
                            prev_group_end[h] = mm.ins

                if cb % 2 == 1:
                    for h in range(2):
                        osb = sb.tile([128, 512], F32, tag=f"osb{cyc}{h}",
                                      name=f"osb{cyc}{h}")
                        nc.vector.tensor_scalar(out=osb[:], in0=outp2[h][:],
                                                scalar1=b2b[:, :], scalar2=None, op0=AL.add)
                        nc.sync.dma_start(out=t_out[8 * cyc + h:8 * cyc + 8:2, :],
                                          in_=osb[0:128:32, :])

    nc.compile()
    return nc


def _pack(a, rows, cols):
    # [k*128, cols] -> [128, k*cols] p-major
    k = rows // 128
    return np.ascontiguousarray(a.reshape(k, 128, cols).transpose(1, 0, 2).reshape(128, k * cols))


def make_in_maps(inputs):
    pack = _pack
    x = np.asarray(inputs["x"], np.float32)
    pred_start = np.asarray(inputs["pred_start"]).astype(np.int64)
    pred_end = np.asarray(inputs["pred_end"]).astype(np.int64)
    pdi = np.asarray(inputs["pred_desc_ids"]).astype(np.int32)
    adi = np.asarray(inputs["arg_desc_ids"]).astype(np.int32)
    label_emb = np.asarray(inputs["label_emb"], np.float32)
    word_emb = np.ascontiguousarray(np.asarray(inputs["word_emb"], np.float32))
    Wa1 = np.asarray(inputs["Wa1"], np.float32)
    ba1 = np.asarray(inputs["ba1"], np.float32)
    Wa2 = np.asarray(inputs["Wa2"], np.float32)
    ba2 = np.asarray(inputs["ba2"], np.float32)
    W1 = np.ascontiguousarray(np.asarray(inputs["W1"], np.float32))
    b1 = np.asarray(inputs["b1"], np.float32)
    import ml_dtypes
    W2c = np.asarray(inputs["W2"], np.float32).reshape(DH)
    W2 = np.zeros((384, 32), np.float32)
    W2[:DH, 0] = W2c
    W2_p = pack(W2, 384, 32).astype(ml_dtypes.bfloat16)
    b2 = np.asarray(inputs["b2"], np.float32)

    c128, c8 = _host_consts()
    wa1_aug = np.zeros((1152, H), np.float32)
    wa1_aug[:1068] = Wa1
    wa1_aug[1068] = ba1
    wa1_p = pack(wa1_aug, 1152, H)
    w1x_p = pack(W1[0:768], 768, DH)
    w1l_p = pack(np.ascontiguousarray(W1[768:1280]), 512, DH)
    w1p_p = pack(np.ascontiguousarray(W1[1196:1964]), 768, DH)
    wa2row = np.ascontiguousarray(Wa2.reshape(1, H))
    b1row = np.ascontiguousarray(b1.reshape(1, DH))
    b2m = np.array([[float(b2[0])]], np.float32)
    ba2m = np.array([[float(ba2[0])]], np.float32)

    in_maps = []
    for core in range(NCORES):
        b, ch = core // 2, core % 2
        ids = adi[b, :, ch * CH:(ch + 1) * CH, :]          # [8, 32, 16]
        flat = ids.reshape(-1)
        aidxn = pack(flat.reshape(NS * CH, LA), NS * CH, LA)
        mat = np.ascontiguousarray(flat.reshape(32, 128).T)  # [128, 32] col g
        aidxg = np.ascontiguousarray(mat[:, GORDER])
        pflat = pdi[b].reshape(-1)
        pidxg = np.ascontiguousarray(pflat.reshape(2, 128).T)
        pse = np.array([[int(pred_start[b]), int(pred_end[b])]], np.int32)
        lembT = np.ascontiguousarray(label_emb[ch * CH:(ch + 1) * CH, :].T)
        in_maps.append({
            "x": pack(x[b], S, H),
            "wemb": word_emb,
            "aidxn": aidxn,
            "aidxg": aidxg,
            "pidxn": np.ascontiguousarray(pdi[b]),
            "pidxg": pidxg,
            "pse": pse,
            "lembT": lembT,
            "wa1": wa1_p,
            "wa2r": wa2row,
            "w1x": w1x_p,
            "w1l": w1l_p,
            "w1p": w1p_p,
            "w2": W2_p,
            "b1r": b1row,
            "b2": b2m,
            "ba2": ba2m,
            "c128": c128,
            "c8": c8,
        })
    return in_maps


def assemble(results):
    logits = np.empty((B, S, C), np.float32)
    for core in range(NCORES):
        b, ch = core // 2, core % 2
        r = results[core]["out"].reshape(CH, S)
        logits[b, :, ch * CH:(ch + 1) * CH] = r.T
    return logits


_NC_CACHE = {}
LAST_RESULTS = None


def kernel(**inputs):
    global LAST_RESULTS
    if "nc" not in _NC_CACHE:
        _NC_CACHE["nc"] = build_program()
    nc = _NC_CACHE["nc"]
    in_maps = make_in_maps(inputs)
    trace = bool(os.environ.get("KBENCH_TRACE"))
    res = run_bass_kernel_spmd(nc, in_maps, core_ids=list(range(NCORES)), trace=trace)
    LAST_RESULTS = res
    return assemble(res.results)

